# revision 8
# baseline (speedup 1.0000x reference)
"""Trainium2 Bass kernel for im2col Conv2d dot-product:
out[b, n] = <enc_x[b, n, :], w_flat> + bias.

Data-parallel over batch: 8 batches per NeuronCore x 8 cores.

TensorEngine split-K formulation (fp16). PSUM cell (m, n) of an
accumulation group sums contributions from column n of EVERY matmul in
the group, so a group of 49 matmuls x 128 rows gives 6272 row-slots per
column position: exactly 128 windows x 49 taps. Window m of column-block
n has its taps spread across the group's matmuls at flat slot
s = m*49 + k -> (matmul t = s//128, row r = s%128):

  stationary_t[r, s//49] = w[s%49]   (s = t*128 + r; one nonzero per row)
  rhs_t[r, n]            = x[window(g, s//49, n), s%49]
  psum[m, n]  +=  over t  ->  full dot of window  g*65536 + m*512 + n

The host pre-arranges x (cast to fp16) so each core reads one flat
[128, 153664] tensor: group-major, then matmul-major, then column --
every DMA is a full-128-partition contiguous load. 6 full groups of
49 matmuls at FD=512 (65536 windows each) + 1 partial group at FD=64.
The Scalar engine drains PSUM -> SBUF with a fused bias add; one
contiguous [128, 512] store per group.

Per core: 39.3 MB fp16 in at the ~360 GB/s HBM-per-NC roofline
(~105 us), PE ~70-100 us, DVE/GpSimd idle. fp16 rounding of x and w
gives rel err ~3e-4 vs the fp32 reference (tolerance 2e-2); products
accumulate in fp32 PSUM.
"""

from contextlib import ExitStack

import numpy as np

import concourse.bass as bass
import concourse.tile as tile
from concourse import mybir

B = 64
WINDOWS = 50176
K = 49
NCORES = 8
BPC = B // NCORES            # batches per core
NWIN = BPC * WINDOWS         # 401408 windows per core

MM_PER_G = 49                # matmuls per PSUM accumulation group
NFULL = 6                    # full groups: 128 x 512 windows each
FULL_N = 512                 # columns (free dim) per full-group matmul
PART_N = 64                  # columns of the final partial group
GROUPW = 128 * FULL_N        # 65536 windows per full group
PARTW = 128 * PART_N         # 8192 windows in the partial group
assert NFULL * GROUPW + PARTW == NWIN

FULL_COLS = MM_PER_G * FULL_N   # 25088 elems per partition per full group
PART_COLS = MM_PER_G * PART_N   # 3136
XCOLS = NFULL * FULL_COLS + PART_COLS  # 153664 fp16 per partition per core

# Full groups load as two chunks of 24 / 25 matmuls for pipelining.
CHUNK_MMS = (24, 25)

FP32 = mybir.dt.float32
FP16 = mybir.dt.float16

_NC = None


def _build_nc():
    nc = bass.Bass(trn_type="TRN2", debug=False, num_devices=NCORES)

    xh = nc.dram_tensor("xh", [128, XCOLS], FP16, kind="ExternalInput").ap()
    ws = nc.dram_tensor("ws", [128, MM_PER_G * 128], FP16,
                        kind="ExternalInput").ap()
    b = nc.dram_tensor("b", [1], FP32, kind="ExternalInput").ap()
    out = nc.dram_tensor("out", [NWIN], FP32, kind="ExternalOutput").ap()

    with tile.TileContext(nc) as tc, ExitStack() as ctx:
        consts = ctx.enter_context(tc.tile_pool(name="consts", bufs=1))
        xpool = ctx.enter_context(tc.tile_pool(name="x", bufs=5))
        pspool = ctx.enter_context(tc.tile_pool(name="ps", bufs=3,
                                                space="PSUM"))
        opool = ctx.enter_context(tc.tile_pool(name="o", bufs=3))

        # consts go on the scalar HWDGE ring: it is idle at startup, so the
        # stationary weights land in ~4 us instead of crawling behind the
        # x-chunk stream (SWDGE via gpsimd measured 85 GB/s -> first matmul
        # stalled until 31 us).
        wst = consts.tile([128, MM_PER_G * 128], FP16)
        nc.scalar.dma_start(out=wst[:], in_=ws)
        bb = consts.tile([128, 1], FP32)
        nc.scalar.dma_start(
            out=bb[:],
            in_=bass.AP(tensor=b.tensor, offset=b.offset,
                        ap=[[0, 128]] + list(b.ap)),
        )

        def load_chunk(col0, ncols, name):
            xt = xpool.tile([128, ncols], FP16, tag="xt", name=name)
            src = bass.AP(tensor=xh.tensor, offset=xh.offset + col0,
                          ap=[[XCOLS, 128], [1, ncols]])
            nc.sync.dma_start(out=xt[:], in_=src)
            return xt

        def drain(ps, g, ncols, name):
            ot = opool.tile([128, ncols], FP32, tag="ot", name=name)
            nc.scalar.activation(
                out=ot[:], in_=ps[:],
                func=mybir.ActivationFunctionType.Identity,
                bias=bb[:, 0:1], scale=1.0,
            )
            dst = bass.AP(tensor=out.tensor,
                          offset=out.offset + g * GROUPW,
                          ap=[[ncols, 128], [1, ncols]])
            nc.scalar.dma_start(out=dst, in_=ot[:])

        for g in range(NFULL):
            gbase = g * FULL_COLS
            chunks = []
            mm0 = 0
            # group 0 ramps with small chunks so the first matmul starts as
            # early as possible; later groups use big chunks for DMA
            # efficiency.
            for ci, cmms in enumerate((7,) * 7 if g == 0 else CHUNK_MMS):
                xt = load_chunk(gbase + mm0 * FULL_N, cmms * FULL_N,
                                f"xt{g}_{ci}")
                chunks.append((mm0, cmms, xt))
                mm0 += cmms
            ps = pspool.tile([128, FULL_N], FP32, tag="ps", name=f"ps{g}")
            for mm0, cmms, xt in chunks:
                for j in range(cmms):
                    t = mm0 + j
                    nc.tensor.matmul(
                        ps[:],
                        lhsT=wst[:, t * 128:(t + 1) * 128],
                        rhs=xt[:, j * FULL_N:(j + 1) * FULL_N],
                        start=(t == 0),
                        stop=(t == MM_PER_G - 1),
                    )
            drain(ps, g, FULL_N, f"ot{g}")

        # partial group: 49 matmuls at FD=64
        xt = load_chunk(NFULL * FULL_COLS, PART_COLS, "xtp")
        ps = pspool.tile([128, PART_N], FP32, tag="psp", name="psp")
        for t in range(MM_PER_G):
            nc.tensor.matmul(
                ps[:],
                lhsT=wst[:, t * 128:(t + 1) * 128],
                rhs=xt[:, t * PART_N:(t + 1) * PART_N],
                start=(t == 0),
                stop=(t == MM_PER_G - 1),
            )
        drain(ps, NFULL, PART_N, "otp")

    return nc


def _split_ctrl_waits(nc, max_waits=1):
    """Work around a walrus codegen limit on this build: instructions accept
    only one sync-wait command. Hoist extra waits onto dedicated no-op
    instructions inserted just before, preserving per-engine order."""
    from concourse import mybir

    for f in nc.m.functions:
        for blk in f.blocks:
            insts = blk.instructions
            i = 0
            while i < len(insts):
                ins = insts[i]
                if (
                    ins.sync_info is not None
                    and len(ins.sync_info.on_wait) > max_waits
                ):
                    waits = list(ins.sync_info.on_wait)
                    keep, extra = waits[:max_waits], waits[max_waits:]
                    ins.sync_info.on_wait = keep
                    for j, wchunk in enumerate(extra):
                        nop = mybir.InstNoOp(
                            name=f"{ins.name}-wsplit{j}",
                            sync_info=mybir.SyncInfo(on_wait=[wchunk], on_update=[]),
                            bass_nofuse=True,
                            engine=ins.engine,
                        )
                        nc.register_instruction(nop, overwrite=True)
                        insts.insert(i, nop)
                        i += 1
                i += 1


def _patch_ldw_opt():
    """Experimental: flip walrus --enable-ldw-opt to true (gated by env)."""
    import os
    import concourse.bass_utils as bu

    if not os.environ.get("KERNEL_LDW_OPT"):
        return
    if getattr(bu, "_ldw_patched", False):
        return
    orig = bu.bir_verify_and_optimise

    def patched(*a, **kw):
        real_run = bu.run_command

        def run2(cmd, **k):
            cmd = ["--enable-ldw-opt=true" if c == "--enable-ldw-opt=false"
                   else c for c in cmd]
            return real_run(cmd, **k)

        bu.run_command = run2
        try:
            return orig(*a, **kw)
        finally:
            bu.run_command = real_run

    bu.bir_verify_and_optimise = patched
    bu._ldw_patched = True


def _get_nc():
    global _NC
    if _NC is None:
        _patch_ldw_opt()
        _NC = _build_nc()
        _split_ctrl_waits(_NC)
    return _NC


def _host_prep(enc_x, weight, bias):
    """Cast to fp16 and pre-arrange per-core tensors for the split-K PE
    formulation (see module docstring for the layout)."""
    xf = np.asarray(enc_x, dtype=np.float32).reshape(NCORES, NWIN, K)
    x16 = xf.astype(np.float16)

    def core_layout(xc):
        parts = []
        for g in range(NFULL):
            xg = xc[g * GROUPW:(g + 1) * GROUPW].reshape(128, FULL_N, K)
            # [m, n, k] -> flat slot s = m*49+k rows: [s, n] -> [t, r, n]
            y = xg.transpose(0, 2, 1).reshape(MM_PER_G * 128, FULL_N)
            z = y.reshape(MM_PER_G, 128, FULL_N).transpose(1, 0, 2)
            parts.append(z.reshape(128, FULL_COLS))
        xp = xc[NFULL * GROUPW:].reshape(128, PART_N, K)
        y = xp.transpose(0, 2, 1).reshape(MM_PER_G * 128, PART_N)
        z = y.reshape(MM_PER_G, 128, PART_N).transpose(1, 0, 2)
        parts.append(z.reshape(128, PART_COLS))
        return np.concatenate(parts, axis=1)

    xh = np.stack([core_layout(x16[i]) for i in range(NCORES)], axis=0)
    xh = np.ascontiguousarray(xh)

    w49 = np.asarray(weight, dtype=np.float32).reshape(K).astype(np.float16)
    ws = np.zeros((128, MM_PER_G * 128), dtype=np.float16)
    s = np.arange(MM_PER_G * 128)
    t, r = s // 128, s % 128
    m, k = s // K, s % K
    ws[r, t * 128 + m] = w49[k]

    bf = np.asarray(bias, dtype=np.float32).reshape(1)
    return xh, ws, bf


def run(enc_x, weight, bias, trace=False, **spmd_kwargs):
    """Run on 8 NeuronCores; returns (out [B, WINDOWS] fp32, BassKernelResults)."""
    from concourse.bass_utils import run_bass_kernel_spmd

    nc = _get_nc()
    xh, ws, bf = _host_prep(enc_x, weight, bias)
    in_maps = [{"xh": xh[i], "ws": ws, "b": bf} for i in range(NCORES)]
    res = run_bass_kernel_spmd(
        nc, in_maps, list(range(NCORES)), trace=trace, **spmd_kwargs
    )
    out = np.stack([res.results[i]["out"] for i in range(NCORES)], axis=0)
    return out.reshape(B, WINDOWS), res


def kernel(enc_x, weight, bias, windows_nb=None):
    out, _ = run(enc_x, weight, bias)
    return out


# revision 11
# speedup vs baseline: 1.1113x; 1.1113x over previous
"""Trainium2 Bass kernel for im2col Conv2d dot-product:
out[b, n] = <enc_x[b, n, :], w_flat> + bias.

Data-parallel over batch: 8 batches per NeuronCore x 8 cores.

TensorEngine split-K formulation (fp16). PSUM cell (m, n) of an
accumulation group sums contributions from column n of EVERY matmul in
the group, so a group of 49 matmuls x 128 rows gives 6272 row-slots per
column position: exactly 128 windows x 49 taps. Window m of column-block
n has its taps spread across the group's matmuls at flat slot
s = m*49 + k -> (matmul t = s//128, row r = s%128):

  stationary_t[r, s//49] = w[s%49]   (s = t*128 + r; one nonzero per row)
  rhs_t[r, n]            = x[window(g, s//49, n), s%49]
  psum[m, n]  +=  over t  ->  full dot of window  g*65536 + m*512 + n

The host pre-arranges x (cast to fp16) so each core reads one flat
[128, 153664] tensor: group-major, then matmul-major, then column --
every DMA is a full-128-partition contiguous load. 6 full groups of
49 matmuls at FD=512 (65536 windows each) + 1 partial group at FD=64.
The Scalar engine drains PSUM -> SBUF with a fused bias add; one
contiguous [128, 512] store per group.

Per core: 39.3 MB fp16 in at the ~360 GB/s HBM-per-NC roofline
(~105 us), PE ~70-100 us, DVE/GpSimd idle. fp16 rounding of x and w
gives rel err ~3e-4 vs the fp32 reference (tolerance 2e-2); products
accumulate in fp32 PSUM.
"""

from contextlib import ExitStack

import numpy as np

import concourse.bass as bass
import concourse.tile as tile
from concourse import mybir

B = 64
WINDOWS = 50176
K = 49
NCORES = 8
BPC = B // NCORES            # batches per core
NWIN = BPC * WINDOWS         # 401408 windows per core

MM_PER_G = 49                # matmuls per PSUM accumulation group
NFULL = 6                    # full groups: 128 x 512 windows each
FULL_N = 512                 # columns (free dim) per full-group matmul
PART_N = 64                  # columns of the final partial group
GROUPW = 128 * FULL_N        # 65536 windows per full group
PARTW = 128 * PART_N         # 8192 windows in the partial group
assert NFULL * GROUPW + PARTW == NWIN

FULL_COLS = MM_PER_G * FULL_N   # 25088 elems per partition per full group
PART_COLS = MM_PER_G * PART_N   # 3136
XCOLS = NFULL * FULL_COLS + PART_COLS  # 153664 fp16 per partition per core

# Per-group chunking (in matmuls). Group 0 ramps with a small first chunk
# so the PE starts early; the last-issued group tapers so the final chunk's
# matmul tail after its DMA lands is short. ~2 MB chunks keep per-DMA
# efficiency high while 8 buffers give ~17 MB of prefetch depth.
CHUNKS_RAMP = (7, 14, 14, 14)
CHUNKS_MID = (16, 16, 17)
CHUNKS_TAIL = (16, 16, 10, 7)
XBUFS = 8

FP32 = mybir.dt.float32
FP16 = mybir.dt.float16

_NC = None


def _build_nc():
    nc = bass.Bass(trn_type="TRN2", debug=False, num_devices=NCORES)

    xh = nc.dram_tensor("xh", [128, XCOLS], FP16, kind="ExternalInput").ap()
    ws = nc.dram_tensor("ws", [128, MM_PER_G * 128], FP16,
                        kind="ExternalInput").ap()
    b = nc.dram_tensor("b", [1], FP32, kind="ExternalInput").ap()
    out = nc.dram_tensor("out", [NWIN], FP32, kind="ExternalOutput").ap()

    with tile.TileContext(nc) as tc, ExitStack() as ctx:
        consts = ctx.enter_context(tc.tile_pool(name="consts", bufs=1))
        xpool = ctx.enter_context(tc.tile_pool(name="x", bufs=XBUFS))
        pspool = ctx.enter_context(tc.tile_pool(name="ps", bufs=3,
                                                space="PSUM"))
        opool = ctx.enter_context(tc.tile_pool(name="o", bufs=3))

        # consts go on the scalar HWDGE ring: it is idle at startup, so the
        # stationary weights land in ~4 us instead of crawling behind the
        # x-chunk stream (SWDGE via gpsimd measured 85 GB/s -> first matmul
        # stalled until 31 us).
        wst = consts.tile([128, MM_PER_G * 128], FP16)
        nc.scalar.dma_start(out=wst[:], in_=ws)
        bb = consts.tile([128, 1], FP32)
        nc.scalar.dma_start(
            out=bb[:],
            in_=bass.AP(tensor=b.tensor, offset=b.offset,
                        ap=[[0, 128]] + list(b.ap)),
        )

        def load_chunk(col0, ncols, name):
            xt = xpool.tile([128, ncols], FP16, tag="xt", name=name)
            src = bass.AP(tensor=xh.tensor, offset=xh.offset + col0,
                          ap=[[XCOLS, 128], [1, ncols]])
            nc.sync.dma_start(out=xt[:], in_=src)
            return xt

        def drain(ps, g, ncols, name):
            ot = opool.tile([128, ncols], FP32, tag="ot", name=name)
            nc.scalar.activation(
                out=ot[:], in_=ps[:],
                func=mybir.ActivationFunctionType.Identity,
                bias=bb[:, 0:1], scale=1.0,
            )
            dst = bass.AP(tensor=out.tensor,
                          offset=out.offset + g * GROUPW,
                          ap=[[ncols, 128], [1, ncols]])
            nc.scalar.dma_start(out=dst, in_=ot[:])

        def full_group(g, chunk_mms):
            gbase = g * FULL_COLS
            chunks = []
            mm0 = 0
            for ci, cmms in enumerate(chunk_mms):
                xt = load_chunk(gbase + mm0 * FULL_N, cmms * FULL_N,
                                f"xt{g}_{ci}")
                chunks.append((mm0, cmms, xt))
                mm0 += cmms
            assert mm0 == MM_PER_G
            ps = pspool.tile([128, FULL_N], FP32, tag="ps", name=f"ps{g}")
            for mm0, cmms, xt in chunks:
                for j in range(cmms):
                    t = mm0 + j
                    nc.tensor.matmul(
                        ps[:],
                        lhsT=wst[:, t * 128:(t + 1) * 128],
                        rhs=xt[:, j * FULL_N:(j + 1) * FULL_N],
                        start=(t == 0),
                        stop=(t == MM_PER_G - 1),
                    )
            drain(ps, g, FULL_N, f"ot{g}")

        def partial_group():
            xt = load_chunk(NFULL * FULL_COLS, PART_COLS, "xtp")
            ps = pspool.tile([128, PART_N], FP32, tag="psp", name="psp")
            for t in range(MM_PER_G):
                nc.tensor.matmul(
                    ps[:],
                    lhsT=wst[:, t * 128:(t + 1) * 128],
                    rhs=xt[:, t * PART_N:(t + 1) * PART_N],
                    start=(t == 0),
                    stop=(t == MM_PER_G - 1),
                )
            drain(ps, NFULL, PART_N, "otp")

        # Issue order: ramp group, the partial group (so its 49 short
        # matmuls are not the kernel tail), mid groups, tapered last group.
        full_group(0, CHUNKS_RAMP)
        partial_group()
        for g in range(1, NFULL - 1):
            full_group(g, CHUNKS_MID)
        full_group(NFULL - 1, CHUNKS_TAIL)

    return nc


def _split_ctrl_waits(nc, max_waits=1):
    """Work around a walrus codegen limit on this build: instructions accept
    only one sync-wait command. Hoist extra waits onto dedicated no-op
    instructions inserted just before, preserving per-engine order."""
    from concourse import mybir

    for f in nc.m.functions:
        for blk in f.blocks:
            insts = blk.instructions
            i = 0
            while i < len(insts):
                ins = insts[i]
                if (
                    ins.sync_info is not None
                    and len(ins.sync_info.on_wait) > max_waits
                ):
                    waits = list(ins.sync_info.on_wait)
                    keep, extra = waits[:max_waits], waits[max_waits:]
                    ins.sync_info.on_wait = keep
                    for j, wchunk in enumerate(extra):
                        nop = mybir.InstNoOp(
                            name=f"{ins.name}-wsplit{j}",
                            sync_info=mybir.SyncInfo(on_wait=[wchunk], on_update=[]),
                            bass_nofuse=True,
                            engine=ins.engine,
                        )
                        nc.register_instruction(nop, overwrite=True)
                        insts.insert(i, nop)
                        i += 1
                i += 1


def _patch_ldw_opt():
    """Experimental: flip walrus --enable-ldw-opt to true (gated by env)."""
    import os
    import concourse.bass_utils as bu

    if not os.environ.get("KERNEL_LDW_OPT"):
        return
    if getattr(bu, "_ldw_patched", False):
        return
    orig = bu.bir_verify_and_optimise

    def patched(*a, **kw):
        real_run = bu.run_command

        def run2(cmd, **k):
            cmd = ["--enable-ldw-opt=true" if c == "--enable-ldw-opt=false"
                   else c for c in cmd]
            return real_run(cmd, **k)

        bu.run_command = run2
        try:
            return orig(*a, **kw)
        finally:
            bu.run_command = real_run

    bu.bir_verify_and_optimise = patched
    bu._ldw_patched = True


def _get_nc():
    global _NC
    if _NC is None:
        _patch_ldw_opt()
        _NC = _build_nc()
        _split_ctrl_waits(_NC)
    return _NC


def _host_prep(enc_x, weight, bias):
    """Cast to fp16 and pre-arrange per-core tensors for the split-K PE
    formulation (see module docstring for the layout)."""
    xf = np.asarray(enc_x, dtype=np.float32).reshape(NCORES, NWIN, K)
    x16 = xf.astype(np.float16)

    def core_layout(xc):
        parts = []
        for g in range(NFULL):
            xg = xc[g * GROUPW:(g + 1) * GROUPW].reshape(128, FULL_N, K)
            # [m, n, k] -> flat slot s = m*49+k rows: [s, n] -> [t, r, n]
            y = xg.transpose(0, 2, 1).reshape(MM_PER_G * 128, FULL_N)
            z = y.reshape(MM_PER_G, 128, FULL_N).transpose(1, 0, 2)
            parts.append(z.reshape(128, FULL_COLS))
        xp = xc[NFULL * GROUPW:].reshape(128, PART_N, K)
        y = xp.transpose(0, 2, 1).reshape(MM_PER_G * 128, PART_N)
        z = y.reshape(MM_PER_G, 128, PART_N).transpose(1, 0, 2)
        parts.append(z.reshape(128, PART_COLS))
        return np.concatenate(parts, axis=1)

    xh = np.stack([core_layout(x16[i]) for i in range(NCORES)], axis=0)
    xh = np.ascontiguousarray(xh)

    w49 = np.asarray(weight, dtype=np.float32).reshape(K).astype(np.float16)
    ws = np.zeros((128, MM_PER_G * 128), dtype=np.float16)
    s = np.arange(MM_PER_G * 128)
    t, r = s // 128, s % 128
    m, k = s // K, s % K
    ws[r, t * 128 + m] = w49[k]

    bf = np.asarray(bias, dtype=np.float32).reshape(1)
    return xh, ws, bf


def run(enc_x, weight, bias, trace=False, **spmd_kwargs):
    """Run on 8 NeuronCores; returns (out [B, WINDOWS] fp32, BassKernelResults)."""
    from concourse.bass_utils import run_bass_kernel_spmd

    nc = _get_nc()
    xh, ws, bf = _host_prep(enc_x, weight, bias)
    in_maps = [{"xh": xh[i], "ws": ws, "b": bf} for i in range(NCORES)]
    res = run_bass_kernel_spmd(
        nc, in_maps, list(range(NCORES)), trace=trace, **spmd_kwargs
    )
    out = np.stack([res.results[i]["out"] for i in range(NCORES)], axis=0)
    return out.reshape(B, WINDOWS), res


def kernel(enc_x, weight, bias, windows_nb=None):
    out, _ = run(enc_x, weight, bias)
    return out
